# revision 26
# baseline (speedup 1.0000x reference)
"""Cross-attention kernel for 8 TRN2 NeuronCores (Bass/Tile).

Reference (fp32):
    q = x @ Wq; k = ctx @ Wk; v = ctx @ Wv        (8 heads, d=64)
    sim = q k^T * d^-0.5 ; attn = softmax(sim)
    out = (attn v) @ Wo + bo

Sharding (data-parallel, no FLOP duplication): core c -> batch c//2,
head-group c%2 (4 heads).  Each core computes a partial [2048, 1024]
output; the host sums the two partials per batch and adds bo.

Per-core dataflow (bf16 matmul operands, fp32 accumulation):
  - host pre-shuffles every input into its exact SBUF image
    ([128, fat-contiguous-run] per partition); loads are striped across
    the three HWDGE queues (sync/scalar/vector) in critical-path order
    so the first exp fires as early as possible
  - QT[d,i] = Wq^T x^T ; KT[d,j] = Wk^T ctx^T ; V[j,:] = [ctx Wv | 1]
    with 64 ones columns per head: the AV matmul then emits the softmax
    denominator pre-broadcast across 64 partitions for free (the AV
    matmul is moving-operand-bound, extra stationary columns are free)
  - simT[j,i] = KT_h @ QT_h (K=64): the two head-pairs of a chunk are
    emitted h0,h64 interleaved so the PE runs them concurrently on
    disjoint row groups (row-tiled matmuls)
  - expT = exp(0.125 simT) on ScalarE (unsafe softmax, |sim|*0.125 < 4)
  - av = [V_h | 1*64]^T @ expT accumulated over j -> [128, i] fp32 PSUM;
    rows 64:128 are the denominator broadcast
  - normalize fully on-chip: reciprocal_approx_fast (custom DVE op) on
    av[64:128], then o2t = av[0:64] * recip on DVE -> exactly the lhsT
    layout the Wo matmul needs.  No DRAM round-trips.
  - out = out2T^T @ Wo, evacuated on DVE (ScalarE does exp only);
    output staged bf16 (host accumulates in fp32)
The kernel is ScalarE-bound (64 exp ops over 8.4M elements) with the PE
stream packed just under it; everything else is arranged to keep the
exp stream dense.
"""

import numpy as np
import ml_dtypes

import concourse.bass as bass
import concourse.tile as tile
from concourse import bacc, mybir
from concourse.bass_utils import run_bass_kernel_spmd

B = 4
I = 2048
J = 1024
FQ = 1024
FC = 768
DH = 64
HPC = 4
DG = HPC * DH      # 256
E = 1024
P = 128
N_CORES = 8
IH = I // 2        # 1024

F32 = mybir.dt.float32
BF16 = mybir.dt.bfloat16

KQ = FQ // P       # 8
KC = FC // P       # 6
TD = DG // P       # 2
JBN = J // P       # 8
ICN = 4            # x i-chunks


def _build():
    nc = bacc.Bacc()
    xt = nc.declare_dram_parameter("xt", [P, KQ * I], BF16, isOutput=False)
    ctxt = nc.declare_dram_parameter("ctxt", [P, KC * J], BF16, isOutput=False)
    wq = nc.declare_dram_parameter("wq", [P, KQ * DG], BF16, isOutput=False)
    wk = nc.declare_dram_parameter("wk", [P, KC * DG], BF16, isOutput=False)
    wv = nc.declare_dram_parameter("wv", [P, KC * DG], BF16, isOutput=False)
    wo = nc.declare_dram_parameter("wo", [P, TD * E], BF16, isOutput=False)
    out = nc.declare_dram_parameter("out", [I, E], BF16, isOutput=True)

    with tile.TileContext(nc) as tc:
        with (
            tc.tile_pool(name="consts", bufs=1) as consts,
            tc.tile_pool(name="expp", bufs=36) as expp,
            tc.tile_pool(name="recp", bufs=2) as recp,
            tc.tile_pool(name="outp", bufs=3) as outp,
            tc.tile_pool(name="pp", bufs=2, space="PSUM") as pp,
            tc.tile_pool(name="pp2", bufs=2, space="PSUM") as pp2,
            tc.tile_pool(name="avp", bufs=1, space="PSUM") as avpool,
        ):
            # ---- PE warm-up: junk matmuls through the whole DMA load
            # phase (~20us) keep the HAM clock-gate open (a cold or
            # re-gated PE runs at half clock for the first real tiles)
            junk = consts.tile([P, 512], BF16, tag="junk")
            nc.vector.memset(junk, 0.0)
            jps = pp2.tile([P, 512], F32, tag="pp2", name="jps")
            for w in range(32):
                nc.tensor.matmul(jps, lhsT=junk[:, 0:P], rhs=junk,
                                 start=True, stop=True)

            def keep_warm(n):
                jp = pp2.tile([P, 512], F32, tag="pp2", name="kw")
                for w in range(n):
                    nc.tensor.matmul(jp, lhsT=junk[:, 0:P], rhs=junk,
                                     start=True, stop=True)

            # ---- loads: striped across the three HWDGE queues
            # (sync/scalar/vector) in critical-path order.  The first
            # exp needs wk+ctx (-> KT) and wq+x0 (-> QT chunk 0).
            wk_sb = consts.tile([P, KC, DG], BF16, tag="wk_sb")
            ctxt_sb = consts.tile([P, KC, J], BF16, tag="ctxt_sb")
            wq_sb = consts.tile([P, KQ, DG], BF16, tag="wq_sb")
            xq_sb = consts.tile([P, ICN, KQ, 512], BF16, tag="xq_sb")
            wv_sb = consts.tile([P, KC, DG], BF16, tag="wv_sb")
            wo_sb = consts.tile([P, TD, E], BF16, tag="wo_sb")

            ctxt_r = ctxt[:, :].rearrange("p (kb j) -> p kb j", kb=KC)

            def load_x(eng, ich, kh):
                # half an i-chunk: kb 0-3 or 4-7 (contiguous 4KB/partition)
                eng.dma_start(
                    out=xq_sb[:, ich, 4 * kh:4 * kh + 4],
                    in_=xt[:, ich * KQ * 512 + kh * 2048:
                           ich * KQ * 512 + kh * 2048 + 2048]
                    .rearrange("p (kb i) -> p kb i", kb=4))

            # striped across sync/scalar HWDGE in critical-path order
            # (first exp needs wk+ctx -> KT and wq+x0,x1 -> QT chunks
            # 0,1); the slow gpsimd SWDGE only gets x3 (needed last)
            nc.sync.dma_start(
                out=wk_sb, in_=wk[:, :].rearrange("p (kb d) -> p kb d", kb=KC))
            nc.sync.dma_start(out=ctxt_sb[:, 0:3], in_=ctxt_r[:, 0:3])
            nc.scalar.dma_start(out=ctxt_sb[:, 3:6], in_=ctxt_r[:, 3:6])
            nc.scalar.dma_start(
                out=wq_sb, in_=wq[:, :].rearrange("p (kb d) -> p kb d", kb=KQ))
            load_x(nc.sync, 0, 0)
            load_x(nc.scalar, 0, 1)
            load_x(nc.sync, 1, 0)
            load_x(nc.scalar, 1, 1)
            load_x(nc.gpsimd, 3, 0)
            load_x(nc.sync, 2, 0)
            load_x(nc.scalar, 2, 1)
            load_x(nc.gpsimd, 3, 1)
            nc.sync.dma_start(
                out=wv_sb, in_=wv[:, :].rearrange("p (kb d) -> p kb d", kb=KC))
            nc.scalar.dma_start(
                out=wo_sb, in_=wo[:, :].rearrange("p (kb e) -> p kb e", kb=TD))

            # ---- projections as emit-functions (most are deferred into
            # the attention schedule as PE filler work)
            kt_sb = [consts.tile([P, J], BF16, tag=f"kt{t}", name=f"kt{t}")
                     for t in range(TD)]

            def emit_kt(t):
                for nch in range(2):
                    ps = pp2.tile([P, 512], F32, tag="pp2", name="ktps")
                    for kb in range(KC):
                        nc.tensor.matmul(
                            ps,
                            lhsT=wk_sb[:, kb, t * P:(t + 1) * P],
                            rhs=ctxt_sb[:, kb, nch * 512:(nch + 1) * 512],
                            start=(kb == 0), stop=(kb == KC - 1),
                        )
                    nc.vector.tensor_copy(
                        kt_sb[t][:, nch * 512:(nch + 1) * 512], ps)

            # V per jb: [128 j, HPC heads, 128]: cols 0:64 = ctx@Wv for
            # the head, cols 64:128 = ones (denominator broadcast rows)
            v_sb = [consts.tile([P, HPC, P], BF16, tag=f"v{jb}",
                                name=f"v{jb}") for jb in range(JBN)]

            def emit_v(jb):
                nc.gpsimd.memset(v_sb[jb], 1.0)
                ps = pp2.tile([P, DG], F32, tag="pp2", name="vps")
                for kb in range(KC):
                    nc.tensor.matmul(
                        ps,
                        lhsT=ctxt_sb[:, kb, jb * P:(jb + 1) * P],
                        rhs=wv_sb[:, kb, :],
                        start=(kb == 0), stop=(kb == KC - 1),
                    )
                nc.vector.tensor_copy(
                    v_sb[jb][:, :, 0:DH],
                    ps.rearrange("p (h d) -> p h d", h=HPC),
                )

            # one tile per (t, i-chunk): sub-tile deps let the first
            # scores run before all of x has even arrived
            qt_sb = [[consts.tile([P, 512], BF16, tag=f"qt{t}{ich}",
                                  name=f"qt{t}{ich}") for ich in range(ICN)]
                     for t in range(TD)]

            def emit_qt(ich, t):
                ps = pp2.tile([P, 512], F32, tag="pp2", name="qtps")
                for kb in range(KQ):
                    nc.tensor.matmul(
                        ps,
                        lhsT=wq_sb[:, kb, t * P:(t + 1) * P],
                        rhs=xq_sb[:, ich, kb, :],
                        start=(kb == 0), stop=(kb == KQ - 1),
                    )
                nc.vector.tensor_copy(qt_sb[t][ich], ps)

            emit_kt(0)
            emit_qt(0, 0)
            emit_qt(1, 0)

            o2t_sb = [[consts.tile([P, IH], BF16, tag=f"o2t{half}{t}",
                                   name=f"o2t{half}{t}")
                       for t in range(TD)] for half in range(2)]

            avtile = [None]

            def emit_av_par(half, hp, par, ets, jbs, csls=(0, 1)):
                for jb in jbs:
                    for nch in csls:
                        csl = slice(nch * 512, (nch + 1) * 512)
                        nc.tensor.matmul(
                            avtile[0][:, csl],
                            lhsT=v_sb[jb][:, 2 * hp + par, :],
                            rhs=ets[par][jb][:, csl],
                            start=(jb == 0), stop=(jb == JBN - 1),
                        )

            def emit_norm(half, hp, par, av=None, csl=slice(0, IH)):
                # av rows 64:128 hold the softmax denominator already
                # broadcast across 64 partitions (ones columns of V).
                # HW quirks: custom DVE ops (recip) only work at base
                # partition 0, and DVE inputs must share a base -- so
                # shift the denominator down with a native copy first.
                if av is None:
                    av = avtile[0]
                den = recp.tile([DH, IH], F32, tag="den", name="den")
                nc.vector.tensor_copy(den[:, csl], av[DH:2 * DH, csl])
                rec = recp.tile([DH, IH], F32, tag="rec", name="rec")
                nc.vector.reciprocal_approx_fast(
                    out=rec[:, csl], in_=den[:, csl])
                nc.vector.tensor_mul(
                    o2t_sb[half][hp][par * DH:par * DH + DH, csl],
                    av[0:DH, csl], rec[:, csl],
                )

            def emit_wo_m(half, m, act_evac=False):
                # psum evacuation on DVE in the body (ScalarE is kept
                # exp-only); in the drain ACT is idle, so evacuate there
                # and leave DVE free for the norm chains.  Out-DMAs
                # alternate between the two HWDGE queues (a single queue
                # serializes the 16 x 256KB stores).
                ot = outp.tile([P, E], BF16, tag="ot", name="ot")
                pss = [pp2.tile([P, 512], F32, tag="pp2",
                                name=f"wopp{n}") for n in range(2)]
                for t in range(TD):
                    for nch in range(2):
                        nc.tensor.matmul(
                            pss[nch],
                            lhsT=o2t_sb[half][t][:, m * P:(m + 1) * P],
                            rhs=wo_sb[:, t, nch * 512:(nch + 1) * 512],
                            start=(t == 0), stop=(t == TD - 1),
                        )
                for nch in range(2):
                    dst = ot[:, nch * 512:(nch + 1) * 512]
                    if act_evac:
                        nc.scalar.activation(
                            out=dst, in_=pss[nch],
                            func=mybir.ActivationFunctionType.Copy)
                    else:
                        nc.vector.tensor_copy(dst, pss[nch])
                r0 = half * IH + m * P
                eng = nc.sync if m % 2 == 0 else nc.scalar
                eng.dma_start(out=out[r0:r0 + P, :], in_=ot)

            # ---- attention schedule: per-jb fine interleave so the
            # in-order PE stream never bursts long enough to starve ACT.
            # extras = deferred PE work (QT chunks, Wo m-blocks) popped
            # between the scores/exp/AV groups.
            pending = None
            for k, (half, hp) in enumerate([(0, 0), (0, 1), (1, 0), (1, 1)]):
                # deferred PE work is spread into k=1/k=2 where the PE
                # otherwise idles under the ACT exp cadence (an idle PE
                # re-gates the HAM clock); emit_v(jb) must be emitted
                # before the AV pop that reads it (Tile builds deps from
                # emission order)
                extras = []
                if k == 0:
                    extras = ([lambda: emit_kt(1),
                               lambda: emit_qt(0, 1), lambda: emit_qt(1, 1)]
                              + [(lambda jb=jb: emit_v(jb))
                                 for jb in range(4)])
                elif k == 1:
                    extras = ([(lambda jb=jb: emit_v(jb))
                               for jb in range(4, JBN)]
                              + [lambda: emit_qt(2, 0), lambda: emit_qt(3, 0)])
                elif k == 2:
                    extras = [lambda: emit_qt(2, 1), lambda: emit_qt(3, 1)]
                elif k == 3:
                    extras = [(lambda m=m: emit_wo_m(0, m))
                              for m in range(6)]
                prev = pending
                if prev is not None:
                    avtile[0] = avpool.tile([P, IH], F32, tag="av",
                                            name="av")
                avq = []
                if prev is not None:
                    avq = ([(0, jb) for jb in range(JBN)]
                           + ["norm0a", "norm0b"]
                           + [(1, jb) for jb in range(JBN)]
                           + ["norm1a", "norm1b"])

                def pop_av():
                    item = avq.pop(0)
                    if isinstance(item, str):
                        par = int(item[4])
                        csl = (slice(0, 512) if item[5] == "a"
                               else slice(512, IH))
                        emit_norm(prev[0], prev[1], par, csl=csl)
                        if item == "norm0b":
                            avtile[0] = avpool.tile([P, IH], F32, tag="av",
                                                    name="av")
                    else:
                        emit_av_par(prev[0], prev[1], item[0], prev[2],
                                    [item[1]])

                t = hp
                ets = [[None] * JBN, [None] * JBN]
                for jb in range(JBN):
                    scs = [pp.tile([P, IH], F32, tag="pp", name=f"sc{par}")
                           for par in range(2)]
                    # h0/h64 interleaved: disjoint PE row groups run
                    # concurrently (row-tiled matmuls)
                    for nch in range(2):
                        for par in range(2):
                            prow = par * DH
                            nc.tensor.matmul(
                                scs[par][:, nch * 512:(nch + 1) * 512],
                                lhsT=kt_sb[t][prow:prow + DH,
                                              jb * P:(jb + 1) * P],
                                rhs=qt_sb[t][half * 2 + nch][prow:prow + DH, :],
                                start=True, stop=True,
                            )
                    for par in range(2):
                        et = expp.tile([P, IH], BF16, tag="et",
                                       name=f"et{par}")
                        if k == 0 and jb < 2:
                            # at the very start, half-width exps fire as
                            # soon as the nch0 scores land (the nch1
                            # scores chase the x1 DMA)
                            for nch in range(2):
                                csl = slice(nch * 512, (nch + 1) * 512)
                                nc.scalar.activation(
                                    out=et[:, csl], in_=scs[par][:, csl],
                                    func=mybir.ActivationFunctionType.Exp,
                                    scale=0.125,
                                )
                        else:
                            nc.scalar.activation(
                                out=et, in_=scs[par],
                                func=mybir.ActivationFunctionType.Exp,
                                scale=0.125,
                            )
                        ets[par][jb] = et
                    for _ in range(3):
                        if avq:
                            pop_av()
                    for _ in range(2):
                        if extras:
                            extras.pop(0)()
                while avq:
                    pop_av()
                while extras:
                    extras.pop(0)()
                if k >= 1:
                    # pair boundary: PE has finished its pair early and
                    # would idle long enough to re-gate the HAM clock
                    keep_warm(3)
                pending = (half, hp, ets)

            # drain the last pair, split by column-half so Wo(1) m-blocks
            # 0-3 start while the second AV column-half still accumulates
            half, hp, ets = pending
            av0 = avpool.tile([P, IH], F32, tag="av", name="av")
            av1 = pp.tile([P, IH], F32, tag="pp", name="av1")
            for jb in range(JBN):
                avtile[0] = av0
                emit_av_par(half, hp, 0, ets, [jb], csls=(0,))
                avtile[0] = av1
                emit_av_par(half, hp, 1, ets, [jb], csls=(0,))
                if jb < 2:
                    emit_wo_m(0, 6 + jb, act_evac=True)
            emit_norm(half, hp, 0, av=av0, csl=slice(0, 512))
            emit_norm(half, hp, 1, av=av1, csl=slice(0, 512))
            keep_warm(2)
            for jb in range(JBN):
                avtile[0] = av0
                emit_av_par(half, hp, 0, ets, [jb], csls=(1,))
                avtile[0] = av1
                emit_av_par(half, hp, 1, ets, [jb], csls=(1,))
                if jb >= 4:
                    emit_wo_m(1, jb - 4, act_evac=True)
            emit_norm(half, hp, 0, av=av0, csl=slice(512, IH))
            emit_norm(half, hp, 1, av=av1, csl=slice(512, IH))
            # keep-warm: the final norms are a DVE-only window; an idle
            # PE would re-gate the HAM clock and run Wo(1) at half speed
            keep_warm(4)
            for m in range(4, 8):
                emit_wo_m(1, m, act_evac=True)

    nc.compile()
    return nc


_NC_CACHE = None


def _get_nc():
    global _NC_CACHE
    if _NC_CACHE is None:
        _NC_CACHE = _build()
    return _NC_CACHE


def _sbuf_image(a):
    """[KB*128, R] row-major -> [128, KB*R]: partition p holds the
    concatenation of rows {kb*128+p} (one contiguous run per partition)."""
    kb = a.shape[0] // P
    return np.ascontiguousarray(
        a.reshape(kb, P, a.shape[1]).transpose(1, 0, 2).reshape(P, -1)
    ).astype(ml_dtypes.bfloat16)


def _x_image(xtb):
    """x^T [1024, 2048] -> per partition: [ich, kb, 512] contiguous."""
    r = xtb.reshape(KQ, P, ICN, 512).transpose(1, 2, 0, 3)
    return np.ascontiguousarray(r.reshape(P, -1)).astype(ml_dtypes.bfloat16)


def _make_in_maps(x, context, Wq, Wk, Wv, Wo):
    in_maps = []
    for c in range(N_CORES):
        b, hg = c // 2, c % 2
        sl = slice(hg * DG, (hg + 1) * DG)
        in_maps.append({
            "xt": _x_image(x[b].T),
            "ctxt": _sbuf_image(context[b].T),
            "wq": _sbuf_image(Wq[:, sl]),
            "wk": _sbuf_image(Wk[:, sl]),
            "wv": _sbuf_image(Wv[:, sl]),
            "wo": _sbuf_image(Wo[sl, :]),
        })
    return in_maps


def _run(inputs, trace=False):
    x = np.asarray(inputs["x"], dtype=np.float32)
    context = np.asarray(inputs["context"], dtype=np.float32)
    Wq = np.asarray(inputs["Wq"], dtype=np.float32)
    Wk = np.asarray(inputs["Wk"], dtype=np.float32)
    Wv = np.asarray(inputs["Wv"], dtype=np.float32)
    Wo = np.asarray(inputs["Wo"], dtype=np.float32)
    bo = np.asarray(inputs["bo"], dtype=np.float32)

    res = run_bass_kernel_spmd(
        _get_nc(), _make_in_maps(x, context, Wq, Wk, Wv, Wo),
        core_ids=list(range(N_CORES)), trace=trace,
    )
    parts = [np.asarray(r["out"], dtype=np.float32) for r in res.results]
    outv = np.stack([parts[2 * b] + parts[2 * b + 1] + bo for b in range(B)])
    return outv.astype(np.float32), res


def kernel(**inputs) -> np.ndarray:
    outv, _ = _run(inputs, trace=False)
    return outv


# revision 27
# speedup vs baseline: 1.1174x; 1.1174x over previous
"""Cross-attention kernel for 8 TRN2 NeuronCores (Bass/Tile).

Reference (fp32):
    q = x @ Wq; k = ctx @ Wk; v = ctx @ Wv        (8 heads, d=64)
    sim = q k^T * d^-0.5 ; attn = softmax(sim)
    out = (attn v) @ Wo + bo

Sharding (data-parallel, no FLOP duplication): core c -> batch c//2,
head-group c%2 (4 heads).  Each core computes a partial [2048, 1024]
output; the host sums the two partials per batch and adds bo.

Per-core dataflow (bf16 matmul operands, fp32 accumulation):
  - host pre-shuffles every input into its exact SBUF image
    ([128, fat-contiguous-run] per partition); loads are striped across
    the three HWDGE queues (sync/scalar/vector) in critical-path order
    so the first exp fires as early as possible
  - QT[d,i] = Wq^T x^T ; KT[d,j] = Wk^T ctx^T ; V[j,:] = [ctx Wv | 1]
    with 64 ones columns per head: the AV matmul then emits the softmax
    denominator pre-broadcast across 64 partitions for free (the AV
    matmul is moving-operand-bound, extra stationary columns are free)
  - simT[j,i] = KT_h @ QT_h (K=64): the two head-pairs of a chunk are
    emitted h0,h64 interleaved so the PE runs them concurrently on
    disjoint row groups (row-tiled matmuls)
  - expT = exp(0.125 simT) on ScalarE (unsafe softmax, |sim|*0.125 < 4)
  - av = [V_h | 1*64]^T @ expT accumulated over j -> [128, i] fp32 PSUM;
    rows 64:128 are the denominator broadcast
  - normalize fully on-chip: reciprocal_approx_fast (custom DVE op) on
    av[64:128], then o2t = av[0:64] * recip on DVE -> exactly the lhsT
    layout the Wo matmul needs.  No DRAM round-trips.
  - out = out2T^T @ Wo, evacuated on DVE (ScalarE does exp only);
    output staged bf16 (host accumulates in fp32)
The kernel is ScalarE-bound (64 exp ops over 8.4M elements) with the PE
stream packed just under it; everything else is arranged to keep the
exp stream dense.
"""

import numpy as np
import ml_dtypes

import concourse.bass as bass
import concourse.tile as tile
from concourse import bacc, mybir
from concourse.bass_utils import run_bass_kernel_spmd

B = 4
I = 2048
J = 1024
FQ = 1024
FC = 768
DH = 64
HPC = 4
DG = HPC * DH      # 256
E = 1024
P = 128
N_CORES = 8
IH = I // 2        # 1024

F32 = mybir.dt.float32
BF16 = mybir.dt.bfloat16

KQ = FQ // P       # 8
KC = FC // P       # 6
TD = DG // P       # 2
JBN = J // P       # 8
ICN = 4            # x i-chunks


def _build():
    nc = bacc.Bacc()
    xt = nc.declare_dram_parameter("xt", [P, KQ * I], BF16, isOutput=False)
    ctxt = nc.declare_dram_parameter("ctxt", [P, KC * J], BF16, isOutput=False)
    wq = nc.declare_dram_parameter("wq", [P, KQ * DG], BF16, isOutput=False)
    wk = nc.declare_dram_parameter("wk", [P, KC * DG], BF16, isOutput=False)
    wv = nc.declare_dram_parameter("wv", [P, KC * DG], BF16, isOutput=False)
    wo = nc.declare_dram_parameter("wo", [P, TD * E], BF16, isOutput=False)
    out = nc.declare_dram_parameter("out", [I, E], BF16, isOutput=True)

    with tile.TileContext(nc) as tc:
        with (
            tc.tile_pool(name="consts", bufs=1) as consts,
            tc.tile_pool(name="expp", bufs=36) as expp,
            tc.tile_pool(name="recp", bufs=2) as recp,
            tc.tile_pool(name="outp", bufs=3) as outp,
            tc.tile_pool(name="pp", bufs=2, space="PSUM") as pp,
            tc.tile_pool(name="pp2", bufs=2, space="PSUM") as pp2,
            tc.tile_pool(name="avp", bufs=1, space="PSUM") as avpool,
        ):
            # ---- PE warm-up: junk matmuls through the whole DMA load
            # phase (~20us) keep the HAM clock-gate open (a cold or
            # re-gated PE runs at half clock for the first real tiles)
            junk = consts.tile([P, 512], BF16, tag="junk")
            nc.vector.memset(junk, 0.0)
            jps = pp2.tile([P, 512], F32, tag="pp2", name="jps")
            for w in range(42):
                nc.tensor.matmul(jps, lhsT=junk[:, 0:P], rhs=junk,
                                 start=True, stop=True)

            def keep_warm(n):
                jp = pp2.tile([P, 512], F32, tag="pp2", name="kw")
                for w in range(n):
                    nc.tensor.matmul(jp, lhsT=junk[:, 0:P], rhs=junk,
                                     start=True, stop=True)

            # ---- loads: striped across the three HWDGE queues
            # (sync/scalar/vector) in critical-path order.  The first
            # exp needs wk+ctx (-> KT) and wq+x0 (-> QT chunk 0).
            wk_sb = consts.tile([P, KC, DG], BF16, tag="wk_sb")
            ctxt_sb = consts.tile([P, KC, J], BF16, tag="ctxt_sb")
            wq_sb = consts.tile([P, KQ, DG], BF16, tag="wq_sb")
            xq_sb = consts.tile([P, ICN, KQ, 512], BF16, tag="xq_sb")
            wv_sb = consts.tile([P, KC, DG], BF16, tag="wv_sb")
            wo_sb = consts.tile([P, TD, E], BF16, tag="wo_sb")

            ctxt_r = ctxt[:, :].rearrange("p (kb j) -> p kb j", kb=KC)

            def load_x(eng, ich, kh):
                # half an i-chunk: kb 0-3 or 4-7 (contiguous 4KB/partition)
                eng.dma_start(
                    out=xq_sb[:, ich, 4 * kh:4 * kh + 4],
                    in_=xt[:, ich * KQ * 512 + kh * 2048:
                           ich * KQ * 512 + kh * 2048 + 2048]
                    .rearrange("p (kb i) -> p kb i", kb=4))

            # striped across sync/scalar HWDGE in critical-path order
            # (first exp needs wk+ctx -> KT and wq+x0,x1 -> QT chunks
            # 0,1); the slow gpsimd SWDGE only gets x3 (needed last)
            nc.sync.dma_start(
                out=wk_sb, in_=wk[:, :].rearrange("p (kb d) -> p kb d", kb=KC))
            nc.scalar.dma_start(
                out=wq_sb, in_=wq[:, :].rearrange("p (kb d) -> p kb d", kb=KQ))
            nc.sync.dma_start(out=ctxt_sb[:, 0:3], in_=ctxt_r[:, 0:3])
            nc.scalar.dma_start(out=ctxt_sb[:, 3:6], in_=ctxt_r[:, 3:6])
            load_x(nc.sync, 0, 0)
            load_x(nc.scalar, 0, 1)
            load_x(nc.sync, 1, 0)
            load_x(nc.scalar, 1, 1)
            load_x(nc.gpsimd, 3, 0)
            load_x(nc.sync, 2, 0)
            load_x(nc.scalar, 2, 1)
            load_x(nc.gpsimd, 3, 1)
            nc.sync.dma_start(
                out=wv_sb, in_=wv[:, :].rearrange("p (kb d) -> p kb d", kb=KC))
            nc.scalar.dma_start(
                out=wo_sb, in_=wo[:, :].rearrange("p (kb e) -> p kb e", kb=TD))

            # ---- projections as emit-functions (most are deferred into
            # the attention schedule as PE filler work)
            kt_sb = [consts.tile([P, J], BF16, tag=f"kt{t}", name=f"kt{t}")
                     for t in range(TD)]

            def emit_kt(t):
                for nch in range(2):
                    ps = pp2.tile([P, 512], F32, tag="pp2", name="ktps")
                    for kb in range(KC):
                        nc.tensor.matmul(
                            ps,
                            lhsT=wk_sb[:, kb, t * P:(t + 1) * P],
                            rhs=ctxt_sb[:, kb, nch * 512:(nch + 1) * 512],
                            start=(kb == 0), stop=(kb == KC - 1),
                        )
                    nc.vector.tensor_copy(
                        kt_sb[t][:, nch * 512:(nch + 1) * 512], ps)

            # V per jb: [128 j, HPC heads, 128]: cols 0:64 = ctx@Wv for
            # the head, cols 64:128 = ones (denominator broadcast rows)
            v_sb = [consts.tile([P, HPC, P], BF16, tag=f"v{jb}",
                                name=f"v{jb}") for jb in range(JBN)]

            def emit_v(jb):
                nc.gpsimd.memset(v_sb[jb], 1.0)
                ps = pp2.tile([P, DG], F32, tag="pp2", name="vps")
                for kb in range(KC):
                    nc.tensor.matmul(
                        ps,
                        lhsT=ctxt_sb[:, kb, jb * P:(jb + 1) * P],
                        rhs=wv_sb[:, kb, :],
                        start=(kb == 0), stop=(kb == KC - 1),
                    )
                nc.vector.tensor_copy(
                    v_sb[jb][:, :, 0:DH],
                    ps.rearrange("p (h d) -> p h d", h=HPC),
                )

            # one tile per (t, i-chunk): sub-tile deps let the first
            # scores run before all of x has even arrived
            qt_sb = [[consts.tile([P, 512], BF16, tag=f"qt{t}{ich}",
                                  name=f"qt{t}{ich}") for ich in range(ICN)]
                     for t in range(TD)]

            def emit_qt(ich, t):
                ps = pp2.tile([P, 512], F32, tag="pp2", name="qtps")
                for kb in range(KQ):
                    nc.tensor.matmul(
                        ps,
                        lhsT=wq_sb[:, kb, t * P:(t + 1) * P],
                        rhs=xq_sb[:, ich, kb, :],
                        start=(kb == 0), stop=(kb == KQ - 1),
                    )
                nc.vector.tensor_copy(qt_sb[t][ich], ps)

            emit_kt(0)
            emit_qt(0, 0)
            emit_qt(1, 0)

            o2t_sb = [[consts.tile([P, IH], BF16, tag=f"o2t{half}{t}",
                                   name=f"o2t{half}{t}")
                       for t in range(TD)] for half in range(2)]

            avtile = [None]

            def emit_av_par(half, hp, par, ets, jbs, csls=(0, 1)):
                for jb in jbs:
                    for nch in csls:
                        csl = slice(nch * 512, (nch + 1) * 512)
                        nc.tensor.matmul(
                            avtile[0][:, csl],
                            lhsT=v_sb[jb][:, 2 * hp + par, :],
                            rhs=ets[par][jb][:, csl],
                            start=(jb == 0), stop=(jb == JBN - 1),
                        )

            def emit_norm(half, hp, par, av=None, csl=slice(0, IH)):
                # av rows 64:128 hold the softmax denominator already
                # broadcast across 64 partitions (ones columns of V).
                # HW quirks: custom DVE ops (recip) only work at base
                # partition 0, and DVE inputs must share a base -- so
                # shift the denominator down with a native copy first.
                if av is None:
                    av = avtile[0]
                den = recp.tile([DH, IH], F32, tag="den", name="den")
                nc.vector.tensor_copy(den[:, csl], av[DH:2 * DH, csl])
                rec = recp.tile([DH, IH], F32, tag="rec", name="rec")
                nc.vector.reciprocal_approx_fast(
                    out=rec[:, csl], in_=den[:, csl])
                nc.vector.tensor_mul(
                    o2t_sb[half][hp][par * DH:par * DH + DH, csl],
                    av[0:DH, csl], rec[:, csl],
                )

            def emit_wo_m(half, m, act_evac=False):
                # psum evacuation on DVE in the body (ScalarE is kept
                # exp-only); in the drain ACT is idle, so evacuate there
                # and leave DVE free for the norm chains.  Out-DMAs
                # alternate between the two HWDGE queues (a single queue
                # serializes the 16 x 256KB stores).
                ot = outp.tile([P, E], BF16, tag="ot", name="ot")
                pss = [pp2.tile([P, 512], F32, tag="pp2",
                                name=f"wopp{n}") for n in range(2)]
                for t in range(TD):
                    for nch in range(2):
                        nc.tensor.matmul(
                            pss[nch],
                            lhsT=o2t_sb[half][t][:, m * P:(m + 1) * P],
                            rhs=wo_sb[:, t, nch * 512:(nch + 1) * 512],
                            start=(t == 0), stop=(t == TD - 1),
                        )
                for nch in range(2):
                    dst = ot[:, nch * 512:(nch + 1) * 512]
                    if act_evac:
                        nc.scalar.activation(
                            out=dst, in_=pss[nch],
                            func=mybir.ActivationFunctionType.Copy)
                    else:
                        nc.vector.tensor_copy(dst, pss[nch])
                r0 = half * IH + m * P
                eng = nc.sync if m % 2 == 0 else nc.scalar
                eng.dma_start(out=out[r0:r0 + P, :], in_=ot)

            # ---- attention schedule: per-jb fine interleave so the
            # in-order PE stream never bursts long enough to starve ACT.
            # extras = deferred PE work (QT chunks, Wo m-blocks) popped
            # between the scores/exp/AV groups.
            pending = None
            for k, (half, hp) in enumerate([(0, 0), (0, 1), (1, 0), (1, 1)]):
                extras = []
                if k == 0:
                    # ALL V projections must be emitted here: the flat AV
                    # queue below reads v_sb[jb] from k==1 on, and Tile
                    # builds deps from emission order (a reader emitted
                    # before its writer reads garbage)
                    extras = ([lambda: emit_kt(1),
                               lambda: emit_qt(0, 1), lambda: emit_qt(1, 1)]
                              + [(lambda jb=jb: emit_v(jb))
                                 for jb in range(JBN)]
                              + [lambda: emit_qt(2, 0), lambda: emit_qt(3, 0)])
                elif k == 1:
                    extras = [lambda: emit_qt(2, 1), lambda: emit_qt(3, 1)]
                elif k == 3:
                    extras = [(lambda m=m: emit_wo_m(0, m))
                              for m in range(4)]
                prev = pending
                if prev is not None:
                    avtile[0] = avpool.tile([P, IH], F32, tag="av",
                                            name="av")
                avq = []
                if prev is not None:
                    avq = ([(0, jb) for jb in range(JBN)]
                           + ["norm0a", "norm0b"]
                           + [(1, jb) for jb in range(JBN)]
                           + ["norm1a", "norm1b"])

                def pop_av():
                    item = avq.pop(0)
                    if isinstance(item, str):
                        par = int(item[4])
                        csl = (slice(0, 512) if item[5] == "a"
                               else slice(512, IH))
                        emit_norm(prev[0], prev[1], par, csl=csl)
                        if item == "norm0b":
                            avtile[0] = avpool.tile([P, IH], F32, tag="av",
                                                    name="av")
                    else:
                        emit_av_par(prev[0], prev[1], item[0], prev[2],
                                    [item[1]])

                t = hp
                ets = [[None] * JBN, [None] * JBN]
                for jb in range(JBN):
                    scs = [pp.tile([P, IH], F32, tag="pp", name=f"sc{par}")
                           for par in range(2)]
                    # h0/h64 interleaved: disjoint PE row groups run
                    # concurrently (row-tiled matmuls)
                    for nch in range(2):
                        for par in range(2):
                            prow = par * DH
                            nc.tensor.matmul(
                                scs[par][:, nch * 512:(nch + 1) * 512],
                                lhsT=kt_sb[t][prow:prow + DH,
                                              jb * P:(jb + 1) * P],
                                rhs=qt_sb[t][half * 2 + nch][prow:prow + DH, :],
                                start=True, stop=True,
                            )
                    for par in range(2):
                        et = expp.tile([P, IH], BF16, tag="et",
                                       name=f"et{par}")
                        nc.scalar.activation(
                            out=et, in_=scs[par],
                            func=mybir.ActivationFunctionType.Exp,
                            scale=0.125,
                        )
                        ets[par][jb] = et
                    for _ in range(3):
                        if avq:
                            pop_av()
                    for _ in range(2):
                        if extras:
                            extras.pop(0)()
                while avq:
                    pop_av()
                while extras:
                    extras.pop(0)()
                pending = (half, hp, ets)

            # drain the last pair, split by column-half so Wo(1) m-blocks
            # 0-3 start while the second AV column-half still accumulates
            half, hp, ets = pending
            av0 = avpool.tile([P, IH], F32, tag="av", name="av")
            av1 = pp.tile([P, IH], F32, tag="pp", name="av1")
            for jb in range(JBN):
                avtile[0] = av0
                emit_av_par(half, hp, 0, ets, [jb], csls=(0,))
                avtile[0] = av1
                emit_av_par(half, hp, 1, ets, [jb], csls=(0,))
                if jb < 4:
                    emit_wo_m(0, 4 + jb, act_evac=True)
            emit_norm(half, hp, 0, av=av0, csl=slice(0, 512))
            emit_norm(half, hp, 1, av=av1, csl=slice(0, 512))
            keep_warm(2)
            for jb in range(JBN):
                avtile[0] = av0
                emit_av_par(half, hp, 0, ets, [jb], csls=(1,))
                avtile[0] = av1
                emit_av_par(half, hp, 1, ets, [jb], csls=(1,))
                if jb >= 4:
                    emit_wo_m(1, jb - 4, act_evac=True)
            emit_norm(half, hp, 0, av=av0, csl=slice(512, IH))
            emit_norm(half, hp, 1, av=av1, csl=slice(512, IH))
            # keep-warm: the final norms are a DVE-only window; an idle
            # PE would re-gate the HAM clock and run Wo(1) at half speed
            keep_warm(4)
            for m in range(4, 8):
                emit_wo_m(1, m, act_evac=True)

    nc.compile()
    return nc


_NC_CACHE = None


def _get_nc():
    global _NC_CACHE
    if _NC_CACHE is None:
        _NC_CACHE = _build()
    return _NC_CACHE


def _sbuf_image(a):
    """[KB*128, R] row-major -> [128, KB*R]: partition p holds the
    concatenation of rows {kb*128+p} (one contiguous run per partition)."""
    kb = a.shape[0] // P
    return np.ascontiguousarray(
        a.reshape(kb, P, a.shape[1]).transpose(1, 0, 2).reshape(P, -1)
    ).astype(ml_dtypes.bfloat16)


def _x_image(xtb):
    """x^T [1024, 2048] -> per partition: [ich, kb, 512] contiguous."""
    r = xtb.reshape(KQ, P, ICN, 512).transpose(1, 2, 0, 3)
    return np.ascontiguousarray(r.reshape(P, -1)).astype(ml_dtypes.bfloat16)


def _make_in_maps(x, context, Wq, Wk, Wv, Wo):
    in_maps = []
    for c in range(N_CORES):
        b, hg = c // 2, c % 2
        sl = slice(hg * DG, (hg + 1) * DG)
        in_maps.append({
            "xt": _x_image(x[b].T),
            "ctxt": _sbuf_image(context[b].T),
            "wq": _sbuf_image(Wq[:, sl]),
            "wk": _sbuf_image(Wk[:, sl]),
            "wv": _sbuf_image(Wv[:, sl]),
            "wo": _sbuf_image(Wo[sl, :]),
        })
    return in_maps


def _run(inputs, trace=False):
    x = np.asarray(inputs["x"], dtype=np.float32)
    context = np.asarray(inputs["context"], dtype=np.float32)
    Wq = np.asarray(inputs["Wq"], dtype=np.float32)
    Wk = np.asarray(inputs["Wk"], dtype=np.float32)
    Wv = np.asarray(inputs["Wv"], dtype=np.float32)
    Wo = np.asarray(inputs["Wo"], dtype=np.float32)
    bo = np.asarray(inputs["bo"], dtype=np.float32)

    res = run_bass_kernel_spmd(
        _get_nc(), _make_in_maps(x, context, Wq, Wk, Wv, Wo),
        core_ids=list(range(N_CORES)), trace=trace,
    )
    parts = [np.asarray(r["out"], dtype=np.float32) for r in res.results]
    outv = np.stack([parts[2 * b] + parts[2 * b + 1] + bo for b in range(B)])
    return outv.astype(np.float32), res


def kernel(**inputs) -> np.ndarray:
    outv, _ = _run(inputs, trace=False)
    return outv
